# revision 14
# baseline (speedup 1.0000x reference)
"""Trainium2 Bass kernel for nn_Attention_86663850099018.

Math (per batch b, reference semantics):
    xn = x_b / ||x_b rows||                      # (N, E) row-normalized
    S  = xn @ xn.T                               # (N, N) cosine scores, symmetric, in [-1, 1]
    P  = softmax(S, axis=1)                      # row softmax over keys
    U  = P @ h_b                                 # (N, H)
    out = U / frob_norm(U over all batches)      # the reference's H* factor cancels

Because S is symmetric and bounded, softmax needs no max subtraction and
E = exp(S) stays symmetric. The column block E[:, Jc] needed as the
stationary (lhsT) operand of the second matmul equals the row block
computed naturally in [i-partition, j-free] layout — the whole pipeline
runs without transposing the score matrix. Rows are relabeled p-major
(row = p*16 + t for SBUF partition p, tile t) so every DRAM<->SBUF
transfer moves 16-32 KiB contiguous per partition.

Sharding: data-parallel over batch B=8, one batch per NeuronCore; the
global Frobenius norm needs one 4-byte AllGather + local reduction.

Matmuls run in fp16 (1 PE cycle/row, fast weight load; |S|<=1 and
exp(S) in [0.37, 2.72] are safely inside fp16 range).
"""

import numpy as np

N, B, E, H = 2048, 8, 256, 512
P = 128
NT = N // P      # 16 row tiles
EC = E // P      # 2 contraction chunks for scores
SF = 512         # matmul free-dim chunk
FC = N // SF     # 4 score chunks per row block
XCH = 4          # x input DMA chunks
TCH2 = 4
NCORES = 8

_CACHE = {}


def _build():
    import concourse.mybir as mybir
    import concourse.tile as tile
    from concourse import bacc
    from concourse.masks import make_identity

    f32 = mybir.dt.float32
    f16 = mybir.dt.float16
    AF = mybir.ActivationFunctionType
    ALU = mybir.AluOpType
    AX = mybir.AxisListType

    nc = bacc.Bacc("TRN2", target_bir_lowering=False, debug=False, num_devices=NCORES)

    x_d = nc.dram_tensor("x", [N, E], f32, kind="ExternalInput").ap()
    h_d = nc.dram_tensor("h", [N, H], f32, kind="ExternalInput").ap()
    o_d = nc.dram_tensor("out", [N, H], f32, kind="ExternalOutput").ap()

    # p-major row relabeling: row = p*NT + t
    x_pt = x_d.rearrange("(p t) e -> p t e", t=NT)
    h_pt = h_d.rearrange("(p t) e -> p t e", t=NT)
    o_pt = o_d.rearrange("(p t) e -> p t e", t=NT)

    with tile.TileContext(nc) as tc:
        with (
            tc.tile_pool(name="const", bufs=1) as constp,
            tc.tile_pool(name="eexpp", bufs=1) as eexpp,
            tc.tile_pool(name="hp", bufs=1) as hp,
            tc.tile_pool(name="zp", bufs=1) as zp,
            tc.tile_pool(name="dramp", bufs=1, space="DRAM") as dramp,
        ):
            ident = constp.tile([P, P], f16)
            make_identity(nc, ident[:])
            ones = constp.tile([P, 1], f32)
            nc.vector.memset(ones[:], 1.0)

            eexp = eexpp.tile([P, NT, N], f16)        # 64 KiB/partition
            h_sb = hp.tile([P, NT, H], f16)           # 16 KiB/partition

            zsum = zp.tile([P, NT], f32)
            zpart = zp.tile([P, 2 * NT], f32)
            zinv = zp.tile([P, NT], f32)
            ssqraw = zp.tile([P, NT], f32)
            ssqw = zp.tile([P, NT], f32)
            ssqcol = zp.tile([P, 1], f32)

            # ---------------- phase 0: load, normalize, transpose ----------
            with tc.tile_pool(name="xntp", bufs=1) as xntp:
                x_all = xntp.tile([P, NT, E], f32)    # 16 KiB/partition
                xnt_ch = [
                    xntp.tile([P, EC, SF], f16, name=f"xnt{q}", tag=f"xnt{q}")
                    for q in range(FC)
                ]                                      # 4 x 2 KiB/partition
                ssq_all = xntp.tile([P, NT], f32)
                lnssq = xntp.tile([P, NT], f32)
                invn = xntp.tile([P, NT], f32)

                with (
                    tc.tile_pool(name="ph0", bufs=3) as ph0,
                    tc.tile_pool(name="psT", bufs=2, space="PSUM") as psT,
                ):
                    TCH = NT // XCH
                    xengs = [nc.sync, nc.scalar, nc.gpsimd, nc.scalar]
                    for ch in range(XCH):
                        t0 = ch * TCH
                        eng = xengs[ch]
                        eng.dma_start(
                            x_all[:, t0 : t0 + TCH, :], x_pt[:, t0 : t0 + TCH, :]
                        )
                        scr = ph0.tile([P, TCH, E], f32, tag="scr")
                        nc.scalar.activation(
                            scr[:], x_all[:, t0 : t0 + TCH, :], AF.Square
                        )
                        nc.vector.tensor_reduce(
                            ssq_all[:, t0 : t0 + TCH],
                            scr[:],
                            axis=AX.X,
                            op=ALU.add,
                        )
                        nc.scalar.activation(
                            lnssq[:, t0 : t0 + TCH],
                            ssq_all[:, t0 : t0 + TCH],
                            AF.Sqrt,
                        )
                        nc.vector.reciprocal(
                            invn[:, t0 : t0 + TCH], lnssq[:, t0 : t0 + TCH]
                        )
                        for t in range(t0, t0 + TCH, 2):
                            pt = psT.tile([P, 2 * EC, P], f16, tag="pt")
                            for u in range(2):
                                xn = ph0.tile([P, E], f16, tag="xn")
                                nc.vector.tensor_scalar_mul(
                                    xn[:],
                                    x_all[:, t + u, :],
                                    invn[:, t + u : t + u + 1],
                                )
                                for c in range(EC):
                                    nc.tensor.transpose(
                                        pt[:, u * EC + c, :],
                                        xn[:, c * P : (c + 1) * P],
                                        ident[:],
                                    )
                            q, tl = t // TCH, t % TCH
                            nc.vector.tensor_copy(
                                xnt_ch[q][:, :, tl * P : (tl + 2) * P].rearrange(
                                    "p c (u k) -> p u c k", k=P
                                ),
                                pt[:].rearrange("p (u c) k -> p u c k", c=EC),
                            )

                # h: big DMA + one fp16 rounding pass (off critical path)
                htmp = xntp.tile([P, NT, H], f32)
                nc.sync.dma_start(htmp[:], h_pt[:])
                nc.vector.tensor_copy(h_sb[:], htmp[:])

                # ---------------- phase A: scores + exp ---------------------
                # psB opens first so phase-B matmuls can interleave into
                # TensorE gaps while ACT paces phase A (4+3 PSUM banks).
                psB_ctx = tc.tile_pool(name="psB", bufs=3, space="PSUM")
                psB = psB_ctx.__enter__()
                with tc.tile_pool(name="psA", bufs=2, space="PSUM") as psA:
                    iq, il = 0, 0
                    for i in range(NT):
                        iq, il = i // TCH2, i % TCH2
                        for half in range(2):
                            ps = psA.tile([P, 2, SF], f32, tag="psA")
                            for c in range(EC):
                                for q in range(2):
                                    jc = half * 2 + q
                                    nc.tensor.matmul(
                                        ps[:, q, :],
                                        xnt_ch[i // 4][
                                            :, c, (i % 4) * P : (i % 4 + 1) * P
                                        ],
                                        xnt_ch[jc][:, c, :],
                                        start=(c == 0),
                                        stop=(c == EC - 1),
                                    )
                            nc.scalar.activation(
                                eexp[:, i, half * 2 * SF : (half + 1) * 2 * SF],
                                ps[:].rearrange("p a b -> p (a b)"),
                                AF.Exp,
                            )
                        nc.vector.tensor_reduce(
                            zsum[:, i : i + 1], eexp[:, i, :], axis=AX.X, op=ALU.add
                        )

            # ---------------- phase B: U_raw = exp(S) @ h -------------------
            with (
                tc.tile_pool(name="outp", bufs=1) as outp,
                tc.tile_pool(name="tailp", bufs=3) as tailp,
            ):
                out_sb = outp.tile([P, NT, H], f32)   # 32 KiB/partition

                nc.vector.reciprocal(zinv[:], zsum[:])
                # preload the sqrt table set during phase B (hidden under PE)
                sqpre = tailp.tile([1, 1], f32, tag="sqpre")
                nc.scalar.activation(sqpre[:], zsum[:1, :1], AF.Sqrt)

                for j in range(NT):
                    ps = psB.tile([P, H], f32, tag="psB")
                    for i in range(NT):
                        nc.tensor.matmul(
                            ps[:],
                            eexp[:, i, j * P : (j + 1) * P],
                            h_sb[:, i, :],
                            start=(i == 0),
                            stop=(i == NT - 1),
                        )
                    # zinv-scaled U to SBUF (DVE); per-row sum of squares
                    nc.vector.tensor_scalar_mul(
                        out_sb[:, j, :], ps[:], zinv[:, j : j + 1]
                    )
                    sqs = tailp.tile([P, H], f32, tag="sqs")
                    nc.scalar.activation(
                        sqs[:],
                        out_sb[:, j, :],
                        AF.Square,
                        accum_out=ssqraw[:, j : j + 1],
                    )

                # ---------------- tail: global norm + writeback -------------
                # partition-reduce ssq via a 1-column fp32 matmul with ones
                nc.vector.tensor_reduce(ssqcol[:], ssqraw[:], axis=AX.X, op=ALU.add)

                psS_ctx = tc.tile_pool(name="psS", bufs=1, space="PSUM")
                psS = psS_ctx.__enter__()
                ps1 = psS.tile([1, 1], f32, tag="ps1")
                nc.tensor.matmul(ps1[:], ones[:], ssqcol[:], start=True, stop=True)
                ss11 = tailp.tile([1, 1], f32, tag="ss11")
                nc.scalar.copy(ss11[:], ps1[:])

                cc_in = dramp.tile([1, 1], f32)
                cc_out = dramp.tile([NCORES, 1], f32)
                nc.gpsimd.dma_start(cc_in[:], ss11[:])
                nc.gpsimd.collective_compute(
                    "AllGather",
                    ALU.bypass,
                    replica_groups=[list(range(NCORES))],
                    ins=[cc_in.opt()],
                    outs=[cc_out.opt()],
                )
                agg = tailp.tile([NCORES, 1], f32, tag="agg")
                nc.sync.dma_start(agg[:], cc_out[:])
                ps2 = psS.tile([1, 1], f32, tag="ps2")
                nc.tensor.matmul(
                    ps2[:], ones[:NCORES, :], agg[:], start=True, stop=True
                )
                sstot = tailp.tile([1, 1], f32, tag="sstot")
                nc.scalar.copy(sstot[:], ps2[:])

                lnt = tailp.tile([1, 1], f32, tag="lnt")
                gsc = tailp.tile([1, 1], f32, tag="gsc")
                nc.scalar.activation(lnt[:], sstot[:], AF.Sqrt)
                nc.vector.reciprocal(gsc[:], lnt[:])
                gbc = tailp.tile([P, 1], f32, tag="gbc")
                nc.gpsimd.partition_broadcast(gbc[:], gsc[:])

                # uniform 1/gnorm scale, batched per 4 blocks, split across
                # DVE and ACT; writeback on both HWDGE rings
                OCH = 4
                engs = [nc.scalar, nc.sync, nc.gpsimd, nc.scalar]
                for ch in range(OCH):
                    j0 = ch * (NT // OCH)
                    blk = out_sb[:, j0 : j0 + NT // OCH, :]
                    if ch % 2 == 0:
                        nc.vector.tensor_scalar_mul(blk, blk, gbc[:])
                    else:
                        nc.scalar.activation(blk, blk, AF.Copy, scale=gbc[:])
                    engs[ch].dma_start(
                        o_pt[:, j0 : j0 + NT // OCH, :],
                        blk,
                    )
                psS_ctx.__exit__(None, None, None)
            psB_ctx.__exit__(None, None, None)

    nc.compile()
    return nc


def _get_nc():
    if "nc" not in _CACHE:
        _CACHE["nc"] = _build()
    return _CACHE["nc"]


def _in_maps(x, h):
    return [
        {
            "x": np.ascontiguousarray(x[:, c, :]),
            "h": np.ascontiguousarray(h[:, c, :]),
        }
        for c in range(NCORES)
    ]


def kernel(x, h):
    from concourse.bass_utils import run_bass_kernel_spmd

    x = np.asarray(x, dtype=np.float32)
    h = np.asarray(h, dtype=np.float32)
    assert x.shape == (N, B, E) and h.shape == (N, B, H)

    nc = _get_nc()
    res = run_bass_kernel_spmd(nc, _in_maps(x, h), core_ids=list(range(NCORES)))
    out = np.empty((N, B, H), dtype=np.float32)
    for c in range(NCORES):
        out[:, c, :] = res.results[c]["out"]
    return out


# Exposed for test.py: run once with tracing to get hardware exec time.
def run_traced(x, h):
    import os
    import shutil

    from concourse.bass_utils import run_bass_kernel_spmd

    x = np.asarray(x, dtype=np.float32)
    h = np.asarray(h, dtype=np.float32)
    nc = _get_nc()
    tdir = "/root/problem/trace_out"
    shutil.rmtree(tdir, ignore_errors=True)
    os.makedirs(tdir, exist_ok=True)
    res = run_bass_kernel_spmd(
        nc, _in_maps(x, h), core_ids=list(range(NCORES)), trace=True, tmpdir=tdir
    )
    out = np.empty((N, B, H), dtype=np.float32)
    for c in range(NCORES):
        out[:, c, :] = res.results[c]["out"]
    return out, res


# revision 15
# speedup vs baseline: 1.0851x; 1.0851x over previous
"""Trainium2 Bass kernel for nn_Attention_86663850099018.

Math (per batch b, reference semantics):
    xn = x_b / ||x_b rows||                      # (N, E) row-normalized
    S  = xn @ xn.T                               # (N, N) cosine scores, symmetric, in [-1, 1]
    P  = softmax(S, axis=1)                      # row softmax over keys
    U  = P @ h_b                                 # (N, H)
    out = U / frob_norm(U over all batches)      # the reference's H* factor cancels

Because S is symmetric and bounded, softmax needs no max subtraction and
E = exp(S) stays symmetric. The column block E[:, Jc] needed as the
stationary (lhsT) operand of the second matmul equals the row block
computed naturally in [i-partition, j-free] layout — the whole pipeline
runs without transposing the score matrix. Rows are relabeled p-major
(row = p*16 + t for SBUF partition p, tile t) so every DRAM<->SBUF
transfer moves 16-32 KiB contiguous per partition.

Sharding: data-parallel over batch B=8, one batch per NeuronCore; the
global Frobenius norm needs one 4-byte AllGather + local reduction.

Matmuls run in fp16 (1 PE cycle/row, fast weight load; |S|<=1 and
exp(S) in [0.37, 2.72] are safely inside fp16 range).
"""

import numpy as np

N, B, E, H = 2048, 8, 256, 512
P = 128
NT = N // P      # 16 row tiles
EC = E // P      # 2 contraction chunks for scores
SF = 512         # matmul free-dim chunk
FC = N // SF     # 4 score chunks per row block
XCH = 4          # x input DMA chunks
TCH2 = 4
NCORES = 8

_CACHE = {}


def _build():
    import concourse.mybir as mybir
    import concourse.tile as tile
    from concourse import bacc
    from concourse.masks import make_identity

    f32 = mybir.dt.float32
    f16 = mybir.dt.float16
    AF = mybir.ActivationFunctionType
    ALU = mybir.AluOpType
    AX = mybir.AxisListType

    nc = bacc.Bacc("TRN2", target_bir_lowering=False, debug=False, num_devices=NCORES)

    x_d = nc.dram_tensor("x", [N, E], f32, kind="ExternalInput").ap()
    h_d = nc.dram_tensor("h", [N, H], f32, kind="ExternalInput").ap()
    o_d = nc.dram_tensor("out", [N, H], f32, kind="ExternalOutput").ap()

    # p-major row relabeling: row = p*NT + t
    x_pt = x_d.rearrange("(p t) e -> p t e", t=NT)
    h_pt = h_d.rearrange("(p t) e -> p t e", t=NT)
    o_pt = o_d.rearrange("(p t) e -> p t e", t=NT)

    with tile.TileContext(nc) as tc:
        with (
            tc.tile_pool(name="const", bufs=1) as constp,
            tc.tile_pool(name="eexpp", bufs=1) as eexpp,
            tc.tile_pool(name="hp", bufs=1) as hp,
            tc.tile_pool(name="zp", bufs=1) as zp,
            tc.tile_pool(name="dramp", bufs=1, space="DRAM") as dramp,
        ):
            ident = constp.tile([P, P], f16)
            make_identity(nc, ident[:])
            ones = constp.tile([P, 1], f32)
            nc.vector.memset(ones[:], 1.0)

            eexp = eexpp.tile([P, NT, N], f16)        # 64 KiB/partition
            h_sb = hp.tile([P, NT, H], f16)           # 16 KiB/partition

            zsum = zp.tile([P, NT], f32)
            zpart = zp.tile([P, 2 * NT], f32)
            zinv = zp.tile([P, NT], f32)
            ssqraw = zp.tile([P, NT], f32)
            ssqw = zp.tile([P, NT], f32)
            ssqcol = zp.tile([P, 1], f32)

            # ---------------- phase 0: load, normalize, transpose ----------
            with tc.tile_pool(name="xntp", bufs=1) as xntp:
                x_all = xntp.tile([P, NT, E], f32)    # 16 KiB/partition
                xnt_ch = [
                    xntp.tile([P, EC, SF], f16, name=f"xnt{q}", tag=f"xnt{q}")
                    for q in range(FC)
                ]                                      # 4 x 2 KiB/partition
                ssq_all = xntp.tile([P, NT], f32)
                lnssq = xntp.tile([P, NT], f32)
                invn = xntp.tile([P, NT], f32)

                with (
                    tc.tile_pool(name="ph0", bufs=3) as ph0,
                    tc.tile_pool(name="psT", bufs=2, space="PSUM") as psT,
                ):
                    TCH = NT // XCH
                    xengs = [nc.sync, nc.scalar, nc.sync, nc.scalar]
                    for ch in range(XCH):
                        t0 = ch * TCH
                        eng = xengs[ch]
                        eng.dma_start(
                            x_all[:, t0 : t0 + TCH, :], x_pt[:, t0 : t0 + TCH, :]
                        )
                        scr = ph0.tile([P, TCH, E], f32, tag="scr")
                        nc.scalar.activation(
                            scr[:], x_all[:, t0 : t0 + TCH, :], AF.Square
                        )
                        nc.vector.tensor_reduce(
                            ssq_all[:, t0 : t0 + TCH],
                            scr[:],
                            axis=AX.X,
                            op=ALU.add,
                        )
                        nc.scalar.activation(
                            lnssq[:, t0 : t0 + TCH],
                            ssq_all[:, t0 : t0 + TCH],
                            AF.Sqrt,
                        )
                        nc.vector.reciprocal(
                            invn[:, t0 : t0 + TCH], lnssq[:, t0 : t0 + TCH]
                        )
                        for t in range(t0, t0 + TCH, 2):
                            pt = psT.tile([P, 2 * EC, P], f16, tag="pt")
                            for u in range(2):
                                xn = ph0.tile([P, E], f16, tag="xn")
                                nc.vector.tensor_scalar_mul(
                                    xn[:],
                                    x_all[:, t + u, :],
                                    invn[:, t + u : t + u + 1],
                                )
                                for c in range(EC):
                                    nc.tensor.transpose(
                                        pt[:, u * EC + c, :],
                                        xn[:, c * P : (c + 1) * P],
                                        ident[:],
                                    )
                            q, tl = t // TCH, t % TCH
                            nc.vector.tensor_copy(
                                xnt_ch[q][:, :, tl * P : (tl + 2) * P].rearrange(
                                    "p c (u k) -> p u c k", k=P
                                ),
                                pt[:].rearrange("p (u c) k -> p u c k", c=EC),
                            )

                # h: big DMA + one fp16 rounding pass (off critical path)
                htmp = xntp.tile([P, NT, H], f32)
                nc.sync.dma_start(htmp[:], h_pt[:])
                nc.vector.tensor_copy(h_sb[:], htmp[:])

                # ---------------- phase A: scores + exp ---------------------
                # psB opens first so phase-B matmuls can interleave into
                # TensorE gaps while ACT paces phase A (4+3 PSUM banks).
                psB_ctx = tc.tile_pool(name="psB", bufs=3, space="PSUM")
                psB = psB_ctx.__enter__()
                with tc.tile_pool(name="psA", bufs=2, space="PSUM") as psA:
                    iq, il = 0, 0
                    for i in range(NT):
                        iq, il = i // TCH2, i % TCH2
                        for half in range(2):
                            ps = psA.tile([P, 2, SF], f32, tag="psA")
                            for c in range(EC):
                                for q in range(2):
                                    jc = half * 2 + q
                                    nc.tensor.matmul(
                                        ps[:, q, :],
                                        xnt_ch[i // 4][
                                            :, c, (i % 4) * P : (i % 4 + 1) * P
                                        ],
                                        xnt_ch[jc][:, c, :],
                                        start=(c == 0),
                                        stop=(c == EC - 1),
                                    )
                            nc.scalar.activation(
                                eexp[:, i, half * 2 * SF : (half + 1) * 2 * SF],
                                ps[:].rearrange("p a b -> p (a b)"),
                                AF.Exp,
                            )
                        nc.vector.tensor_reduce(
                            zsum[:, i : i + 1], eexp[:, i, :], axis=AX.X, op=ALU.add
                        )

            # ---------------- phase B: U_raw = exp(S) @ h -------------------
            with (
                tc.tile_pool(name="outp", bufs=1) as outp,
                tc.tile_pool(name="tailp", bufs=3) as tailp,
            ):
                out_sb = outp.tile([P, NT, H], f32)   # 32 KiB/partition

                nc.vector.reciprocal(zinv[:], zsum[:])
                # preload the sqrt table set during phase B (hidden under PE)
                sqpre = tailp.tile([1, 1], f32, tag="sqpre")
                nc.scalar.activation(sqpre[:], zsum[:1, :1], AF.Sqrt)

                for j in range(NT):
                    ps = psB.tile([P, H], f32, tag="psB")
                    for i in range(NT):
                        nc.tensor.matmul(
                            ps[:],
                            eexp[:, i, j * P : (j + 1) * P],
                            h_sb[:, i, :],
                            start=(i == 0),
                            stop=(i == NT - 1),
                        )
                    # zinv-scaled U to SBUF (DVE); per-row sum of squares
                    nc.vector.tensor_scalar_mul(
                        out_sb[:, j, :], ps[:], zinv[:, j : j + 1]
                    )
                    sqs = tailp.tile([P, H], f32, tag="sqs")
                    nc.scalar.activation(
                        sqs[:],
                        out_sb[:, j, :],
                        AF.Square,
                        accum_out=ssqraw[:, j : j + 1],
                    )

                # ---------------- tail: global norm + writeback -------------
                # partition-reduce ssq via a 1-column fp32 matmul with ones
                nc.vector.tensor_reduce(ssqcol[:], ssqraw[:], axis=AX.X, op=ALU.add)

                psS_ctx = tc.tile_pool(name="psS", bufs=1, space="PSUM")
                psS = psS_ctx.__enter__()
                ps1 = psS.tile([1, 1], f32, tag="ps1")
                nc.tensor.matmul(ps1[:], ones[:], ssqcol[:], start=True, stop=True)
                ss11 = tailp.tile([1, 1], f32, tag="ss11")
                nc.scalar.copy(ss11[:], ps1[:])

                cc_in = dramp.tile([1, 1], f32)
                cc_out = dramp.tile([NCORES, 1], f32)
                nc.gpsimd.dma_start(cc_in[:], ss11[:])
                nc.gpsimd.collective_compute(
                    "AllGather",
                    ALU.bypass,
                    replica_groups=[list(range(NCORES))],
                    ins=[cc_in.opt()],
                    outs=[cc_out.opt()],
                )
                agg = tailp.tile([NCORES, 1], f32, tag="agg")
                nc.sync.dma_start(agg[:], cc_out[:])
                ps2 = psS.tile([1, 1], f32, tag="ps2")
                nc.tensor.matmul(
                    ps2[:], ones[:NCORES, :], agg[:], start=True, stop=True
                )
                sstot = tailp.tile([1, 1], f32, tag="sstot")
                nc.scalar.copy(sstot[:], ps2[:])

                lnt = tailp.tile([1, 1], f32, tag="lnt")
                gsc = tailp.tile([1, 1], f32, tag="gsc")
                nc.scalar.activation(lnt[:], sstot[:], AF.Sqrt)
                nc.vector.reciprocal(gsc[:], lnt[:])
                gbc = tailp.tile([P, 1], f32, tag="gbc")
                nc.gpsimd.partition_broadcast(gbc[:], gsc[:])

                # uniform 1/gnorm scale, batched per 4 blocks, split across
                # DVE and ACT; writeback on both HWDGE rings
                OCH = 4
                engs = [nc.scalar, nc.sync, nc.gpsimd, nc.scalar]
                for ch in range(OCH):
                    j0 = ch * (NT // OCH)
                    blk = out_sb[:, j0 : j0 + NT // OCH, :]
                    if ch % 2 == 0:
                        nc.vector.tensor_scalar_mul(blk, blk, gbc[:])
                    else:
                        nc.scalar.activation(blk, blk, AF.Copy, scale=gbc[:])
                    engs[ch].dma_start(
                        o_pt[:, j0 : j0 + NT // OCH, :],
                        blk,
                    )
                psS_ctx.__exit__(None, None, None)
            psB_ctx.__exit__(None, None, None)

    nc.compile()
    return nc


def _get_nc():
    if "nc" not in _CACHE:
        _CACHE["nc"] = _build()
    return _CACHE["nc"]


def _in_maps(x, h):
    return [
        {
            "x": np.ascontiguousarray(x[:, c, :]),
            "h": np.ascontiguousarray(h[:, c, :]),
        }
        for c in range(NCORES)
    ]


def kernel(x, h):
    from concourse.bass_utils import run_bass_kernel_spmd

    x = np.asarray(x, dtype=np.float32)
    h = np.asarray(h, dtype=np.float32)
    assert x.shape == (N, B, E) and h.shape == (N, B, H)

    nc = _get_nc()
    res = run_bass_kernel_spmd(nc, _in_maps(x, h), core_ids=list(range(NCORES)))
    out = np.empty((N, B, H), dtype=np.float32)
    for c in range(NCORES):
        out[:, c, :] = res.results[c]["out"]
    return out


# Exposed for test.py: run once with tracing to get hardware exec time.
def run_traced(x, h):
    import os
    import shutil

    from concourse.bass_utils import run_bass_kernel_spmd

    x = np.asarray(x, dtype=np.float32)
    h = np.asarray(h, dtype=np.float32)
    nc = _get_nc()
    tdir = "/root/problem/trace_out"
    shutil.rmtree(tdir, ignore_errors=True)
    os.makedirs(tdir, exist_ok=True)
    res = run_bass_kernel_spmd(
        nc, _in_maps(x, h), core_ids=list(range(NCORES)), trace=True, tmpdir=tdir
    )
    out = np.empty((N, B, H), dtype=np.float32)
    for c in range(NCORES):
        out[:, c, :] = res.results[c]["out"]
    return out, res
